# revision 28
# baseline (speedup 1.0000x reference)
"""Trainium2 Bass kernel for a GPT-2 style transformer block (B=4, T=2048, C=768, H=12).

Sharding: core pair (2b, 2b+1) owns batch row b.

- Attention is head-split tensor-parallel (6 heads per core) over the full
  row; each core produces a partial attention projection for all 2048
  tokens.  TWO pairwise ReduceScatters (one per own-token 512-chunk,
  issued as soon as their inputs exist, q processed in order [0,2,1,3])
  hand each core the summed attention output for ITS half of the tokens
  while later attention chunks still compute.
- Everything downstream (residual, LN2, FFN with the full 3072 hidden dim,
  residual2, output) is per-token and runs on each core's own 1024-token
  half, pipelined per 512-token chunk with zero further communication.

Device layout is feature-major ("transposed"): the residual stream lives
as x^T [C, T] so every matmul contraction dim (C or hidden) is on SBUF
partitions and no on-device transposes are ever needed.

LayerNorm gains/biases are folded into the following matmul weights on the
host (W' = diag(ln_w) @ W, b' = ln_b @ W + b), so the device only applies
the per-token affine (x*A + B) with A = 1/(sqrt(var)+eps) via one ScalarE
Sqrt + a DVE reciprocal, keeping the ScalarE activation-table switches to
~5 for the whole kernel (sqrt set <-> exp set <-> gelu set).

Attention is flash-style with S^T = K^T.T @ Q^T blocks in [s,q] layout.
Heads are processed in PAIRS: the two 64-contraction S matmuls of a pair
auto-tile to PE rows 0-63 / 64-127 and run concurrently, writing one
[128,1024] PSUM pair tile that a single batched exp evicts.  No max
subtraction (scores provably tiny at this scale); 1/sqrt(64) folded into
the exp scale; P summed via a ones-column appended to V so the softmax
denominator falls out of the PV matmul.

DMA count is minimized (~60 per core vs ~300 naive): packed weight /
bias / broadcast / collective transfers, since each DMA costs ~0.6us of
shared descriptor-generation bandwidth.
"""

import os
import sys

for _p in ("/opt/trn_rl_repo", "/root/.axon_site/_ro/trn_rl_repo"):
    if os.path.isdir(_p) and _p not in sys.path:
        sys.path.append(_p)

import ml_dtypes
import numpy as np

import concourse.bass as bass
import concourse.mybir as mybir
import concourse.tile as tile
from concourse import bacc
from concourse.vector_clock import ScopedClock

F32 = mybir.dt.float32
BF16 = mybir.dt.bfloat16
AF = mybir.ActivationFunctionType
ALU = mybir.AluOpType

B, T, C = 4, 2048, 768
H, D = 12, 64
HID = 3072
EPS = 1e-6
N_CORES = 8
TH = T // 2            # own token half

CT = C // 128          # 6 c-chunks
HL = H // 2            # 6 heads per core
HPAIR = HL // 2        # 3 head pairs per core
HCT = HID // 128       # 24 hidden chunks
QC = T // 512          # 4 col-chunks of 512 over the full row
NEG = -1.0e9

# biaspack column layout (f32 [128, NB])
BP_BQ = 0      # 3 cols: q bias chunks
BP_BK = 3      # 3 cols: k bias chunks
BP_BAP = 6     # 6 cols: attn proj bias (already /2)
BP_BMP = 12    # 6 cols: mlp proj bias
BP_BFC = 18    # 24 cols: fc bias
NB = 42

# ---------------------------------------------------------------------------
# Tile's final drain carries one sem-wait per logical processor; the walrus
# in this container only encodes 1 sync wait per CTRL instruction.  Spread
# the extras over SP nops.
_MAXW = 1


def _patched_drain_and_barrier(self, tick_clock, wait_clock):
    nc = self.nc
    drain_inst = nc.sync.drain()
    wait_clock.add_sem_waits(
        drain_inst.ins, ScopedClock({None: tick_clock.global_clock})
    )
    si = drain_inst.ins.sync_info
    if si is not None and si.on_wait and len(si.on_wait) > _MAXW:
        waits = list(si.on_wait)
        si.on_wait = waits[:_MAXW]
        rest = waits[_MAXW:]
        while rest:
            nop = nc.sync.nop(nofuse=True, hint="drain_split")
            nsi = nop.ins.sync_info
            if nsi is None:
                nop.ins.sync_info = mybir.SyncInfo(
                    on_wait=rest[:_MAXW], on_update=[]
                )
            else:
                nsi.on_wait = rest[:_MAXW]
            rest = rest[_MAXW:]
    nc.all_engine_barrier()
    assert self.sems is not None
    popped = nc._tile_sem_poison_stack.pop()
    assert popped is self._sem_poison
    nc.clear_and_free_semaphores(list(self.sems.allocated().values()))
    nc.all_engine_barrier()


tile.TileContext._drain_and_barrier = _patched_drain_and_barrier


def _bcast_ap(t, p, n):
    """Partition-stride-0 AP over DRAM tensor/tile t: read rows linearly as
    one n-element run, replicated to p partitions."""
    return bass.AP(tensor=t.tensor, offset=t.offset, ap=[[0, p], [1, n]])


def _rowgrp_ap(t, nrow, ngrp, ncol, col_off=0, row_len=None):
    """DRAM AP viewing t (2D [ngrp*nrow, row_len]) as [nrow, ngrp, ncol]:
    partition dim = row within group, then group, then cols."""
    if row_len is None:
        row_len = ncol
    return bass.AP(
        tensor=t.tensor,
        offset=t.offset + col_off,
        ap=[[row_len, nrow], [row_len * nrow, ngrp], [1, ncol]],
    )


def build_nc(reps=1, fake_cc=False, **tune):
    _t = dict(s_ps=2, o_ps=1, p_ps=2, att_sc=3, h1p=2, fc_ps=3, mp_ps=3,
              qkv_ps=3, v_ps=2, sc1=3)
    _t.update(tune)
    nc = bacc.Bacc(None, target_bir_lowering=False, debug=False, num_devices=N_CORES)

    xTb = nc.declare_dram_parameter("xTb", [C, T], BF16, isOutput=False)
    xh = nc.declare_dram_parameter("xh", [C, TH], F32, isOutput=False)
    Wqkv = nc.declare_dram_parameter("Wqkv", [C, 1152], BF16, isOutput=False)
    Wp = nc.declare_dram_parameter("Wp", [384, C], BF16, isOutput=False)
    Wfc = nc.declare_dram_parameter("Wfc", [C, HID], BF16, isOutput=False)
    Wmp = nc.declare_dram_parameter("Wmp", [HID, C], BF16, isOutput=False)
    biasp = nc.declare_dram_parameter("biasp", [128, NB], F32, isOutput=False)
    bv = nc.declare_dram_parameter("bv", [384], F32, isOutput=False)
    maskT = nc.declare_dram_parameter("maskT", [128, 128], F32, isOutput=False)
    outT = nc.declare_dram_parameter("outT", [C, TH], F32, isOutput=True)

    # chunked pairwise ReduceScatter buffers: chunk a covers own tokens
    # 0:512 (slots q=0 / q=2), chunk b covers own tokens 512:1024 (q=1/q=3)
    arin = [nc.dram_tensor(f"arin_{i}", [2, C, 512], BF16) for i in range(2)]
    arout = [
        nc.dram_tensor(f"arout_{i}", [C, 512], BF16) for i in range(2)
    ]
    groups = [[2 * i, 2 * i + 1] for i in range(4)]

    for _rep in range(reps):
        with tile.TileContext(nc) as tc:
            with (
                tc.tile_pool(name="consts", bufs=1) as consts,
                tc.tile_pool(name="small", bufs=4) as small,
                tc.tile_pool(name="bc", bufs=3) as bc,
                tc.tile_pool(name="persist", bufs=1) as persist,
                tc.tile_pool(name="wfcp", bufs=1) as wfcp,
                tc.tile_pool(name="dramp", bufs=2, space="DRAM") as dramp,
            ):
                ones_b = consts.tile([128, 1], BF16, tag="ones", name="ones")
                nc.vector.memset(ones_b, 1.0)
                mask_sb = consts.tile([128, 128], F32, tag="mask", name="mask")
                nc.sync.dma_start(out=mask_sb, in_=maskT[:, :])
                bp_sb = consts.tile([128, NB], F32, tag="bp", name="bp")
                nc.sync.dma_start(out=bp_sb, in_=biasp[:, :])
                bv_b = consts.tile([128, 384], F32, tag="bvb", name="bvb")
                nc.sync.dma_start(out=bv_b, in_=_bcast_ap(bv[:], 128, 384))

                # x residual (own half) resident: [128, 6*1024] f32
                xh_all = persist.tile([128, CT * TH], F32, tag="xh", name="xh")

                # x2 = x + attn residual (own half), f32 resident
                # x2 = x + attn residual, bf16, c-chunk-major [c0 1024 | c1 ...]
                x2a = persist.tile([128, CT * TH], BF16, tag="x2a", name="x2a")

                def x2_ap(hn, width=512):
                    return bass.AP(
                        tensor=x2a.tensor,
                        offset=x2a.offset + hn * 512,
                        ap=[list(x2a.ap[0]), [TH, CT], [1, width]],
                    )


                def ln_stats(n_tag, stats_ps, xb_src, nsl, ab_dst):
                    """Emit stats for one 512-token chunk.

                    xb_src(c) -> bf16 [128, 512] slice of LN input chunk c.
                    ab_dst: bc-pool tile [128, 1024] bf16 receiving the
                    broadcast A (cols 0:512) and B (cols 512:1024).
                    """
                    ps = stats_ps.tile([33, 512], F32, tag="lnst", name="lnst")
                    for c in range(CT):
                        xbs = xb_src(c)
                        xs = small.tile(
                            [128, 512], BF16, tag="ln_xs", name="ln_xs", bufs=3
                        )
                        nc.vector.tensor_mul(out=xs, in0=xbs, in1=xbs)
                        nc.tensor.matmul(
                            ps[0:1, :], ones_b, xbs,
                            start=(c == 0), stop=(c == CT - 1),
                        )
                        nc.tensor.matmul(
                            ps[32:33, :], ones_b, xs,
                            start=(c == 0), stop=(c == CT - 1),
                        )
                    mean = small.tile([1, 512], F32, tag="ln_mean",
                                      name="ln_mean", bufs=2)
                    ex2 = small.tile([1, 512], F32, tag="ln_ex2", name="ln_ex2",
                                     bufs=2)
                    nc.scalar.activation(out=mean, in_=ps[0:1, :],
                                         func=AF.Copy, scale=1.0 / C)
                    nc.scalar.activation(out=ex2, in_=ps[32:33, :],
                                         func=AF.Copy, scale=1.0 / C)
                    var = small.tile([1, 512], F32, tag="ln_var", name="ln_var",
                                     bufs=2)
                    nc.vector.tensor_mul(out=var, in0=mean, in1=mean)
                    nc.vector.tensor_sub(out=var, in0=ex2, in1=var)
                    a_row = small.tile([1, 512], BF16, tag="ln_a", name="ln_a",
                                       bufs=2)
                    b_row = small.tile([1, 512], BF16, tag="ln_b", name="ln_b",
                                       bufs=2)
                    std = small.tile([1, 512], F32, tag="ln_std", name="ln_std",
                                     bufs=2)
                    nc.scalar.activation(out=std, in_=var, func=AF.Sqrt)
                    nc.vector.tensor_scalar_add(out=std, in0=std, scalar1=EPS)
                    with nc.allow_low_precision(reason="LN scale to bf16"):
                        nc.vector.reciprocal(out=a_row, in_=std)
                    # B = -mean * A
                    nc.vector.scalar_tensor_tensor(
                        out=b_row, in0=mean, scalar=-1.0,
                        in1=a_row, op0=ALU.mult, op1=ALU.mult,
                    )
                    dab = dramp.tile([2, 512], BF16, tag=f"d_ab{n_tag}",
                                     name=f"d_ab{n_tag}", bufs=2)
                    nc.sync.dma_start(out=dab[0:1, :], in_=a_row)
                    nc.sync.dma_start(out=dab[1:2, :], in_=b_row)
                    nc.sync.dma_start(out=ab_dst, in_=_bcast_ap(dab, 128, 1024))

                def ln_apply(scratch, xb_slice, ab_b, out_t):
                    """out_t[128,512] bf16 = xb*A + B."""
                    t1 = scratch.tile([128, 512], BF16, tag="ln_t1", name="ln_t1")
                    nc.vector.tensor_mul(out=t1, in0=xb_slice, in1=ab_b[:, 0:512])
                    nc.vector.tensor_add(out=out_t, in0=t1, in1=ab_b[:, 512:1024])

                # ============== Phase A: load x, LN1 stats, QKV ==============
                with tc.tile_pool(name="attin", bufs=1) as attin:
                    QT = [
                        attin.tile([128, T], BF16, tag=f"QT{c}", name=f"QT{c}")
                        for c in range(3)
                    ]
                    KT = [
                        attin.tile([128, T], BF16, tag=f"KT{c}", name=f"KT{c}")
                        for c in range(3)
                    ]
                    V = [
                        attin.tile([128, HL, 65], BF16, tag=f"V{t}", name=f"V{t}")
                        for t in range(T // 128)
                    ]
                    for t in range(T // 128):
                        nc.vector.memset(V[t][:, :, 64:65], 1.0)

                    with (
                        tc.tile_pool(name="xbp", bufs=1) as xbp,
                        tc.tile_pool(name="wqkvp", bufs=1) as wqkvp,
                        tc.tile_pool(name="h1p", bufs=_t["h1p"]) as h1p,
                        tc.tile_pool(name="sc1", bufs=_t["sc1"]) as sc1,
                        tc.tile_pool(name="st1_ps", bufs=2, space="PSUM") as st1_ps,
                        tc.tile_pool(name="qkv_ps", bufs=_t["qkv_ps"], space="PSUM") as qkv_ps,
                        tc.tile_pool(name="v_ps", bufs=_t["v_ps"], space="PSUM") as v_ps,
                    ):
                        # resident bf16 x (full row), loaded directly
                        xb = [
                            xbp.tile([128, T], BF16, tag=f"xb{c}", name=f"xb{c}")
                            for c in range(CT)
                        ]
                        for c in range(CT):
                            nc.sync.dma_start(
                                out=xb[c], in_=xTb[c * 128 : (c + 1) * 128, :]
                            )
                        wqkv_sb = [
                            wqkvp.tile([128, 1152], BF16, tag=f"wqkv{c}",
                                       name=f"wqkv{c}")
                            for c in range(CT)
                        ]
                        for c in range(CT):
                            nc.sync.dma_start(
                                out=wqkv_sb[c],
                                in_=Wqkv[c * 128 : (c + 1) * 128, :],
                            )

                        nc.sync.dma_start(
                            out=xh_all, in_=_rowgrp_ap(xh[:, :], 128, CT, TH)
                        )
                        ab1 = [
                            bc.tile([128, 1024], BF16, tag=f"ab1_{n}",
                                    name=f"ab1_{n}", bufs=1)
                            for n in range(QC)
                        ]

                        def emit_stats(n):
                            nsl = slice(n * 512, (n + 1) * 512)
                            ln_stats(
                                "1", st1_ps, lambda c: xb[c][:, nsl], nsl, ab1[n]
                            )

                        def emit_qkv(n):
                            nsl = slice(n * 512, (n + 1) * 512)
                            h1c = []
                            for c in range(CT):
                                hh = h1p.tile([128, 512], BF16, tag=f"h1c{c}",
                                              name=f"h1c{c}")
                                ln_apply(sc1, xb[c][:, nsl], ab1[n], hh)
                                h1c.append(hh)
                            for bi, dst in ((0, QT), (1, KT)):
                                for oc in range(3):
                                    ps = qkv_ps.tile([128, 512], F32, tag="qkv",
                                                     name="qkv")
                                    for c in range(CT):
                                        nc.tensor.matmul(
                                            ps,
                                            wqkv_sb[c][
                                                :, bi * 384 + oc * 128
                                                : bi * 384 + (oc + 1) * 128
                                            ],
                                            h1c[c],
                                            start=(c == 0),
                                            stop=(c == CT - 1),
                                        )
                                    nc.scalar.activation(
                                        out=dst[oc][:, nsl], in_=ps,
                                        func=AF.Identity,
                                        bias=bp_sb[:, (BP_BQ, BP_BK)[bi] + oc : (BP_BQ, BP_BK)[bi] + oc + 1],
                                    )
                            for tl in range(4):
                                t = n * 4 + tl
                                ps = v_ps.tile([128, 384], F32, tag="vps",
                                               name="vps")
                                for c in range(CT):
                                    nc.tensor.matmul(
                                        ps,
                                        h1c[c][:, tl * 128 : (tl + 1) * 128],
                                        wqkv_sb[c][:, 768:1152],
                                        start=(c == 0),
                                        stop=(c == CT - 1),
                                    )
                                nc.vector.tensor_add(
                                    out=V[t][:, :, 0:64],
                                    in0=ps.rearrange("p (h d) -> p h d", h=HL),
                                    in1=bv_b.rearrange("p (h d) -> p h d", h=HL),
                                )

                        # software pipeline: stats run ahead of qkv
                        emit_stats(0)
                        emit_stats(1)
                        emit_qkv(0)
                        emit_stats(2)
                        emit_stats(3)
                        emit_qkv(1)
                        emit_qkv(2)
                        emit_qkv(3)

                    # ============== Phase B: attention + proj + RS ==========
                    with (
                        tc.tile_pool(name="wpp", bufs=1) as wpp,
                        tc.tile_pool(name="scp", bufs=2) as scp,
                        tc.tile_pool(name="att_s_ps", bufs=_t["s_ps"], space="PSUM") as s_ps,
                        tc.tile_pool(name="att_o_ps", bufs=_t["o_ps"], space="PSUM") as o_ps,
                        tc.tile_pool(name="proj_ps", bufs=_t["p_ps"], space="PSUM") as p_ps,
                        tc.tile_pool(name="att_sc", bufs=_t["att_sc"]) as att_sc,
                        tc.tile_pool(name="yraw", bufs=2) as yraw_p,
                    ):
                        wp_sb = wpp.tile([128, 3 * C], BF16, tag="wp", name="wp")
                        nc.sync.dma_start(
                            out=wp_sb, in_=_rowgrp_ap(Wp[:, :], 128, 3, C)
                        )
                        wfc_sb = [
                            wfcp.tile([128, HID], BF16, tag=f"wfc{c}",
                                      name=f"wfc{c}")
                            for c in range(CT)
                        ]
                        for c in range(CT):
                            nc.sync.dma_start(
                                out=wfc_sb[c],
                                in_=Wfc[c * 128 : (c + 1) * 128, :],
                            )

                        yT = [
                            att_sc.tile([128, 512], BF16, tag=f"yT{c}",
                                        name=f"yT{c}", bufs=2)
                            for c in range(3)
                        ]

                        for q in (0, 2, 1, 3):
                            qsl = slice(q * 512, (q + 1) * 512)
                            nst = 4 * q + 4
                            rv = att_sc.tile([1, HL * 512], BF16, tag="rv",
                                             name="rv", bufs=2)
                            ypair = [
                                yraw_p.tile([128, 512], BF16, tag=f"yp{p}",
                                            name=f"yp{p}")
                                for p in range(HPAIR)
                            ]
                            for ht in range(HPAIR):
                                po_e = o_ps.tile([65, 512], F32, tag="poe",
                                                 name="poe")
                                po_o = o_ps.tile([65, 512], F32, tag="poo",
                                                 name="poo")
                                for st in range(nst):
                                    r = st - 4 * q
                                    qlo = 128 * r if r >= 0 else 0
                                    csl = slice(qlo, 512)
                                    osl = slice(512 + qlo, 1024)
                                    sp = s_ps.tile([128, 1024], F32, tag="sp",
                                                   name="sp")
                                    ssl = slice(st * 128, (st + 1) * 128)
                                    nc.tensor.matmul(
                                        sp[:, csl],
                                        KT[ht][0:64, ssl],
                                        QT[ht][0:64, q * 512 + qlo : (q + 1) * 512],
                                        start=True, stop=True,
                                    )
                                    nc.tensor.matmul(
                                        sp[:, osl],
                                        KT[ht][64:128, ssl],
                                        QT[ht][64:128, q * 512 + qlo : (q + 1) * 512],
                                        start=True, stop=True,
                                    )
                                    pt = att_sc.tile([128, 1024], BF16, tag="pt",
                                                     name="pt")
                                    if r >= 0:
                                        spv = sp[:, 128 * r :].rearrange(
                                            "p (g x) -> p g x", g=2, x=128,
                                            allow_nc=True,
                                        ) if False else bass.AP(
                                            tensor=sp.tensor,
                                            offset=sp.offset + 128 * r,
                                            ap=[list(sp.ap[0]), [512, 2],
                                                [1, 128]],
                                        )
                                        mk2 = bass.AP(
                                            tensor=mask_sb.tensor,
                                            offset=mask_sb.offset,
                                            ap=[list(mask_sb.ap[0]), [0, 2],
                                                [1, 128]],
                                        )
                                        nc.vector.tensor_add(
                                            out=spv, in0=spv, in1=mk2,
                                        )
                                    if r <= 0:
                                        nc.scalar.activation(
                                            out=pt, in_=sp, func=AF.Exp,
                                            scale=0.125,
                                        )
                                    else:
                                        spv = bass.AP(
                                            tensor=sp.tensor,
                                            offset=sp.offset + qlo,
                                            ap=[list(sp.ap[0]), [512, 2],
                                                [1, 512 - qlo]],
                                        )
                                        ptv = bass.AP(
                                            tensor=pt.tensor,
                                            offset=pt.offset + qlo,
                                            ap=[list(pt.ap[0]), [512, 2],
                                                [1, 512 - qlo]],
                                        )
                                        nc.scalar.activation(
                                            out=ptv, in_=spv,
                                            func=AF.Exp, scale=0.125,
                                        )
                                    nc.tensor.matmul(
                                        po_e[:, csl],
                                        V[st][:, 2 * ht, :],
                                        pt[:, csl],
                                        start=(st == 0), stop=(st == nst - 1),
                                    )
                                    nc.tensor.matmul(
                                        po_o[:, csl],
                                        V[st][:, 2 * ht + 1, :],
                                        pt[:, osl],
                                        start=(st == 0), stop=(st == nst - 1),
                                    )
                                for par, po in ((0, po_e), (1, po_o)):
                                    h = 2 * ht + par
                                    with nc.allow_low_precision(
                                        reason="softmax denom recip to bf16"
                                    ):
                                        nc.vector.reciprocal(
                                            out=rv[0:1, h * 512 : (h + 1) * 512],
                                            in_=po[64:65, :],
                                        )
                                    nc.vector.tensor_copy(
                                        out=ypair[ht][par * 64 : par * 64 + 64, :],
                                        in_=po[0:64, :],
                                    )
                            drv = dramp.tile([1, HL * 512], BF16, tag="d_rv",
                                             name="d_rv", bufs=2)
                            nc.sync.dma_start(out=drv, in_=rv)
                            rb = att_sc.tile([128, HL * 512], BF16, tag="rb",
                                             name="rb", bufs=2)
                            nc.sync.dma_start(
                                out=rb, in_=_bcast_ap(drv, 128, HL * 512)
                            )
                            for ht in range(HPAIR):
                                for par in range(2):
                                    h = 2 * ht + par
                                    psl = slice(par * 64, par * 64 + 64)
                                    nc.vector.tensor_mul(
                                        out=yT[ht][psl, :],
                                        in0=ypair[ht][psl, :],
                                        in1=rb[psl, h * 512 : (h + 1) * 512],
                                    )
                            approj = scp.tile([128, CT * 512], BF16, tag="apj",
                                              name="apj")
                            for oc in range(CT):
                                ps = p_ps.tile([128, 512], F32, tag="pps",
                                               name="pps")
                                for ci in range(3):
                                    nc.tensor.matmul(
                                        ps,
                                        wp_sb[:, ci * C + oc * 128
                                              : ci * C + (oc + 1) * 128],
                                        yT[ci],
                                        start=(ci == 0), stop=(ci == 2),
                                    )
                                nc.vector.tensor_scalar_add(
                                    out=approj[:, oc * 512 : (oc + 1) * 512],
                                    in0=ps,
                                    scalar1=bp_sb[:, BP_BAP + oc : BP_BAP + oc + 1],
                                )
                            nc.sync.dma_start(
                                out=_rowgrp_ap(arin[q % 2][q // 2, :, :], 128, CT, 512),
                                in_=approj,
                            )
                            if q == 2 or q == 3:
                                i = q % 2
                                if fake_cc:
                                    nc.sync.dma_start(
                                        out=arout[i][:, :], in_=arin[i][0, :, :]
                                    )
                                else:
                                    nc.gpsimd.collective_compute(
                                        "ReduceScatter",
                                        ALU.add,
                                        replica_groups=groups,
                                        ins=[arin[i][:, :, :]],
                                        outs=[arout[i][:, :]],
                                    )

                # ========= Phase C/D: residual + LN2 + FFN per half-chunk ====
                with (
                    tc.tile_pool(name="ffnw", bufs=1) as ffnw,
                    tc.tile_pool(name="gtp", bufs=1) as gtp,
                    tc.tile_pool(name="scr", bufs=2) as scr,
                    tc.tile_pool(name="sc3", bufs=3) as sc3,
                    tc.tile_pool(name="st2_ps", bufs=2, space="PSUM") as st2_ps,
                    tc.tile_pool(name="fc_ps", bufs=_t["fc_ps"], space="PSUM") as fc_ps,
                    tc.tile_pool(name="mp_ps", bufs=_t["mp_ps"], space="PSUM") as mp_ps,
                ):
                    # wmp_sb[g] holds m-chunks 4g..4g+3 side by side
                    wmp_sb = [
                        ffnw.tile([128, 4 * C], BF16, tag=f"wmp{g}", name=f"wmp{g}")
                        for g in range(6)
                    ]
                    for g in range(6):
                        nc.sync.dma_start(
                            out=wmp_sb[g],
                            in_=_rowgrp_ap(
                                Wmp[g * 512 : (g + 1) * 512, :], 128, 4, C
                            ),
                        )
                    gT = [
                        gtp.tile([128, 512], BF16, tag=f"gT{m}", name=f"gT{m}")
                        for m in range(HCT)
                    ]
                    for hn in range(2):
                        nsl = slice(hn * 512, (hn + 1) * 512)
                        att_all = scr.tile([128, CT * 512], BF16, tag="attall",
                                           name="attall")
                        nc.sync.dma_start(
                            out=att_all,
                            in_=_rowgrp_ap(arout[hn][:, :], 128, CT, 512),
                        )
                        xh_v = bass.AP(
                            tensor=xh_all.tensor,
                            offset=xh_all.offset + hn * 512,
                            ap=[list(xh_all.ap[0]), [TH, CT], [1, 512]],
                        )
                        nc.vector.tensor_add(
                            out=x2_ap(hn), in0=xh_v, in1=att_all,
                        )
                        ab2 = bc.tile([128, 1024], BF16, tag="ab2", name="ab2")
                        ln_stats(
                            "2", st2_ps,
                            lambda c, _n=nsl: x2a[:, c * TH + _n.start
                                                  : c * TH + _n.stop],
                            nsl, ab2,
                        )
                        h2c = []
                        for c in range(CT):
                            hh = sc3.tile([128, 512], BF16, tag=f"h2c{c}",
                                          name=f"h2c{c}", bufs=2)
                            ln_apply(
                                sc3,
                                x2a[:, c * TH + nsl.start : c * TH + nsl.stop],
                                ab2, hh,
                            )
                            h2c.append(hh)
                        for m in range(HCT):
                            ps = fc_ps.tile([128, 512], F32, tag="fps",
                                            name="fps")
                            for c in range(CT):
                                nc.tensor.matmul(
                                    ps,
                                    wfc_sb[c][:, m * 128 : (m + 1) * 128],
                                    h2c[c],
                                    start=(c == 0), stop=(c == CT - 1),
                                )
                            nc.scalar.activation(
                                out=gT[m], in_=ps, func=AF.Gelu,
                                bias=bp_sb[:, BP_BFC + m : BP_BFC + m + 1],
                            )
                        for oc in range(CT):
                            ps = mp_ps.tile([128, 512], F32, tag="mps",
                                            name="mps")
                            for m in range(HCT):
                                nc.tensor.matmul(
                                    ps,
                                    wmp_sb[m // 4][
                                        :, (m % 4) * C + oc * 128
                                        : (m % 4) * C + (oc + 1) * 128
                                    ],
                                    gT[m],
                                    start=(m == 0), stop=(m == HCT - 1),
                                )
                            mp = sc3.tile([128, 512], F32, tag="mp_ev",
                                          name="mp_ev", bufs=2)
                            nc.scalar.activation(
                                out=mp, in_=ps, func=AF.Identity,
                                bias=bp_sb[:, BP_BMP + oc : BP_BMP + oc + 1],
                            )
                            o = sc3.tile([128, 512], F32, tag="r2o",
                                         name="r2o", bufs=2)
                            nc.vector.tensor_add(
                                out=o,
                                in0=x2a[:, oc * TH + nsl.start
                                        : oc * TH + nsl.stop],
                                in1=mp,
                            )
                            nc.sync.dma_start(
                                out=outT[oc * 128 : (oc + 1) * 128, nsl], in_=o
                            )

    nc.finalize()
    return nc


# ---------------------------------------------------------------------------
_RUNNER = {}
_NC = None


def _get_nc():
    global _NC
    if _NC is None:
        _NC = build_nc()
    return _NC


def _make_runner(chain=1, nc=None):
    import jax
    from jax.sharding import Mesh, PartitionSpec
    from jax.experimental.shard_map import shard_map
    from concourse import bass2jax

    if nc is None:
        nc = _get_nc()
    bass2jax.install_neuronx_cc_hook()

    partition_name = (
        nc.partition_id_tensor.name if nc.partition_id_tensor else None
    )
    in_names, out_names, out_avals, zero_outs = [], [], [], []
    for alloc in nc.m.functions[0].allocations:
        if not isinstance(alloc, mybir.MemoryLocationSet):
            continue
        name = alloc.memorylocations[0].name
        if alloc.kind == "ExternalInput":
            if name != partition_name:
                in_names.append(name)
        elif alloc.kind == "ExternalOutput":
            shape = tuple(alloc.tensor_shape)
            dtype = mybir.dt.np(alloc.dtype)
            out_names.append(name)
            out_avals.append(jax.core.ShapedArray(shape, dtype))
            zero_outs.append(np.zeros(shape, dtype))
    n_params = len(in_names)
    n_outs = len(out_avals)
    all_names = in_names + out_names
    if partition_name is not None:
        all_names = all_names + [partition_name]
    donate = tuple(range(n_params, n_params + n_outs))

    def _body(*args):
        operands = list(args)
        if partition_name is not None:
            operands.append(bass2jax.partition_id_tensor())
        outs = bass2jax._bass_exec_p.bind(
            *operands,
            out_avals=tuple(out_avals),
            in_names=tuple(all_names),
            out_names=tuple(out_names),
            lowering_input_output_aliases=(),
            sim_require_finite=True,
            sim_require_nnan=True,
            nc=nc,
        )
        return tuple(outs)

    devices = jax.devices()[:N_CORES]
    mesh = Mesh(np.asarray(devices), ("core",))
    in_specs = (PartitionSpec("core"),) * (n_params + n_outs)
    out_specs = (PartitionSpec("core"),) * n_outs
    sharded = jax.jit(
        shard_map(
            _body, mesh=mesh, in_specs=in_specs, out_specs=out_specs, check_rep=False
        ),
        donate_argnums=donate,
        keep_unused=True,
    )
    return sharded, in_names, out_names, out_avals, zero_outs


def get_runner(chain=1):
    if chain not in _RUNNER:
        _RUNNER[chain] = _make_runner(chain)
    return _RUNNER[chain]


def make_core_inputs(
    x, ln1_w, ln1_b, W_attn, b_attn, W_attn_proj, b_attn_proj,
    ln2_w, ln2_b, W_fc, b_fc, W_mlp_proj, b_mlp_proj,
):
    """Host-side sharding: returns list of 8 dicts of per-core numpy arrays."""
    bf = ml_dtypes.bfloat16
    x = np.asarray(x, np.float32)
    ln1_w = np.asarray(ln1_w, np.float32)
    ln1_b = np.asarray(ln1_b, np.float32)
    ln2_w = np.asarray(ln2_w, np.float32)
    ln2_b = np.asarray(ln2_b, np.float32)
    W_attn = np.asarray(W_attn, np.float32)
    b_attn = np.asarray(b_attn, np.float32)
    W_fc = np.asarray(W_fc, np.float32)
    b_fc = np.asarray(b_fc, np.float32)

    srow, scol = np.meshgrid(np.arange(128), np.arange(128), indexing="ij")
    maskT = np.where(srow <= scol, 0.0, NEG).astype(np.float32)

    # fold LN1 gain into W_attn, LN1 bias into b_attn
    Wattn_f = W_attn * ln1_w[:, None]
    battn_f = b_attn + ln1_b @ W_attn
    # fold LN2 gain into W_fc, bias into b_fc
    Wfc_f = (W_fc * ln2_w[:, None]).astype(bf)
    bfc_f = b_fc + ln2_b @ W_fc
    wmp_bf = np.ascontiguousarray(W_mlp_proj).astype(bf)

    core_ins = []
    for core in range(N_CORES):
        b, par = core // 2, core % 2
        hs = slice(par * 384, (par + 1) * 384)
        ks = slice(C + par * 384, C + (par + 1) * 384)
        vs = slice(2 * C + par * 384, 2 * C + (par + 1) * 384)
        xt = np.ascontiguousarray(x[b].T)
        xtb = xt.astype(bf)

        Wqkv = np.concatenate(
            [Wattn_f[:, hs], Wattn_f[:, ks], Wattn_f[:, vs]], axis=1
        ).astype(bf)

        biasp = np.zeros((128, NB), np.float32)
        biasp[:, BP_BQ : BP_BQ + 3] = battn_f[hs].reshape(3, 128).T
        biasp[:, BP_BK : BP_BK + 3] = battn_f[ks].reshape(3, 128).T
        biasp[:, BP_BAP : BP_BAP + 6] = (
            np.asarray(b_attn_proj, np.float32).reshape(6, 128).T / 2
        )
        biasp[:, BP_BMP : BP_BMP + 6] = (
            np.asarray(b_mlp_proj, np.float32).reshape(6, 128).T
        )
        biasp[:, BP_BFC : BP_BFC + 24] = bfc_f.reshape(24, 128).T

        core_ins.append(
            dict(
                xTb=xtb,
                xh=np.ascontiguousarray(xt[:, par * TH : (par + 1) * TH]),
                Wqkv=Wqkv,
                Wp=np.ascontiguousarray(W_attn_proj[hs, :]).astype(bf),
                Wfc=Wfc_f,
                Wmp=wmp_bf,
                biasp=biasp,
                bv=battn_f[vs].astype(np.float32),
                maskT=maskT,
            )
        )
    return core_ins


def run_cores(core_ins):
    """Execute the SPMD program; returns [N_CORES, C, TH] stacked outT."""
    sharded, in_names, out_names, out_avals, zero_outs = get_runner()
    concat_in = [
        np.concatenate([np.asarray(core_ins[c][n]) for c in range(N_CORES)], axis=0)
        for n in in_names
    ]
    concat_zeros = [
        np.zeros((N_CORES * z.shape[0], *z.shape[1:]), z.dtype) for z in zero_outs
    ]
    outs = sharded(*concat_in, *concat_zeros)
    return np.asarray(outs[0]).reshape(N_CORES, C, TH)


def kernel(**inputs):
    core_ins = make_core_inputs(**inputs)
    o = run_cores(core_ins)
    out = np.empty((B, T, C), np.float32)
    for b in range(B):
        out[b, 0:TH] = o[2 * b].T
        out[b, TH:] = o[2 * b + 1].T
    return out


# revision 30
# speedup vs baseline: 1.0705x; 1.0705x over previous
"""Trainium2 Bass kernel for a GPT-2 style transformer block (B=4, T=2048, C=768, H=12).

Sharding: core pair (2b, 2b+1) owns batch row b.

- Attention is head-split tensor-parallel (6 heads per core) over the full
  row; each core produces a partial attention projection for all 2048
  tokens.  TWO pairwise ReduceScatters (one per own-token 512-chunk,
  issued as soon as their inputs exist, q processed in order [0,2,1,3])
  hand each core the summed attention output for ITS half of the tokens
  while later attention chunks still compute.
- Everything downstream (residual, LN2, FFN with the full 3072 hidden dim,
  residual2, output) is per-token and runs on each core's own 1024-token
  half, pipelined per 512-token chunk with zero further communication.

Device layout is feature-major ("transposed"): the residual stream lives
as x^T [C, T] so every matmul contraction dim (C or hidden) is on SBUF
partitions and no on-device transposes are ever needed.

LayerNorm gains/biases are folded into the following matmul weights on the
host (W' = diag(ln_w) @ W, b' = ln_b @ W + b), so the device only applies
the per-token affine (x*A + B) with A = 1/(sqrt(var)+eps) via one ScalarE
Sqrt + a DVE reciprocal, keeping the ScalarE activation-table switches to
~5 for the whole kernel (sqrt set <-> exp set <-> gelu set).

Attention is flash-style with S^T = K^T.T @ Q^T blocks in [s,q] layout.
Heads are processed in PAIRS: the two 64-contraction S matmuls of a pair
auto-tile to PE rows 0-63 / 64-127 and run concurrently, writing one
[128,1024] PSUM pair tile that a single batched exp evicts.  No max
subtraction (scores provably tiny at this scale); 1/sqrt(64) folded into
the exp scale; P summed via a ones-column appended to V so the softmax
denominator falls out of the PV matmul.

DMA count is minimized (~60 per core vs ~300 naive): packed weight /
bias / broadcast / collective transfers, since each DMA costs ~0.6us of
shared descriptor-generation bandwidth.
"""

import os
import sys

for _p in ("/opt/trn_rl_repo", "/root/.axon_site/_ro/trn_rl_repo"):
    if os.path.isdir(_p) and _p not in sys.path:
        sys.path.append(_p)

import ml_dtypes
import numpy as np

import concourse.bass as bass
import concourse.mybir as mybir
import concourse.tile as tile
from concourse import bacc
from concourse.vector_clock import ScopedClock

F32 = mybir.dt.float32
BF16 = mybir.dt.bfloat16
AF = mybir.ActivationFunctionType
ALU = mybir.AluOpType

B, T, C = 4, 2048, 768
H, D = 12, 64
HID = 3072
EPS = 1e-6
N_CORES = 8
TH = T // 2            # own token half

CT = C // 128          # 6 c-chunks
HL = H // 2            # 6 heads per core
HPAIR = HL // 2        # 3 head pairs per core
HCT = HID // 128       # 24 hidden chunks
QC = T // 512          # 4 col-chunks of 512 over the full row
NEG = -1.0e9

# biaspack column layout (f32 [128, NB])
BP_BQ = 0      # 3 cols: q bias chunks
BP_BK = 3      # 3 cols: k bias chunks
BP_BAP = 6     # 6 cols: attn proj bias (already /2)
BP_BMP = 12    # 6 cols: mlp proj bias
BP_BFC = 18    # 24 cols: fc bias
NB = 42

# ---------------------------------------------------------------------------
# Tile's final drain carries one sem-wait per logical processor; the walrus
# in this container only encodes 1 sync wait per CTRL instruction.  Spread
# the extras over SP nops.
_MAXW = 1


def _patched_drain_and_barrier(self, tick_clock, wait_clock):
    nc = self.nc
    drain_inst = nc.sync.drain()
    wait_clock.add_sem_waits(
        drain_inst.ins, ScopedClock({None: tick_clock.global_clock})
    )
    si = drain_inst.ins.sync_info
    if si is not None and si.on_wait and len(si.on_wait) > _MAXW:
        waits = list(si.on_wait)
        si.on_wait = waits[:_MAXW]
        rest = waits[_MAXW:]
        while rest:
            nop = nc.sync.nop(nofuse=True, hint="drain_split")
            nsi = nop.ins.sync_info
            if nsi is None:
                nop.ins.sync_info = mybir.SyncInfo(
                    on_wait=rest[:_MAXW], on_update=[]
                )
            else:
                nsi.on_wait = rest[:_MAXW]
            rest = rest[_MAXW:]
    nc.all_engine_barrier()
    assert self.sems is not None
    popped = nc._tile_sem_poison_stack.pop()
    assert popped is self._sem_poison
    nc.clear_and_free_semaphores(list(self.sems.allocated().values()))
    nc.all_engine_barrier()


tile.TileContext._drain_and_barrier = _patched_drain_and_barrier


def _bcast_ap(t, p, n):
    """Partition-stride-0 AP over DRAM tensor/tile t: read rows linearly as
    one n-element run, replicated to p partitions."""
    return bass.AP(tensor=t.tensor, offset=t.offset, ap=[[0, p], [1, n]])


def _rowgrp_ap(t, nrow, ngrp, ncol, col_off=0, row_len=None):
    """DRAM AP viewing t (2D [ngrp*nrow, row_len]) as [nrow, ngrp, ncol]:
    partition dim = row within group, then group, then cols."""
    if row_len is None:
        row_len = ncol
    return bass.AP(
        tensor=t.tensor,
        offset=t.offset + col_off,
        ap=[[row_len, nrow], [row_len * nrow, ngrp], [1, ncol]],
    )


def build_nc(reps=1, fake_cc=False, **tune):
    _t = dict(s_ps=2, o_ps=1, p_ps=2, att_sc=3, h1p=2, fc_ps=3, mp_ps=3,
              qkv_ps=3, v_ps=2, sc1=3)
    _t.update(tune)
    nc = bacc.Bacc(None, target_bir_lowering=False, debug=False, num_devices=N_CORES)

    xTb = nc.declare_dram_parameter("xTb", [C, T], BF16, isOutput=False)
    xh = nc.declare_dram_parameter("xh", [C, TH], F32, isOutput=False)
    Wqkv = nc.declare_dram_parameter("Wqkv", [C, 1152], BF16, isOutput=False)
    Wp = nc.declare_dram_parameter("Wp", [384, C], BF16, isOutput=False)
    Wfc = nc.declare_dram_parameter("Wfc", [C, HID], BF16, isOutput=False)
    Wmp = nc.declare_dram_parameter("Wmp", [HID, C], BF16, isOutput=False)
    biasp = nc.declare_dram_parameter("biasp", [128, NB], F32, isOutput=False)
    bv = nc.declare_dram_parameter("bv", [384], F32, isOutput=False)
    maskT = nc.declare_dram_parameter("maskT", [128, 128], F32, isOutput=False)
    outT = nc.declare_dram_parameter("outT", [C, TH], F32, isOutput=True)

    # chunked pairwise ReduceScatter buffers: chunk a covers own tokens
    # 0:512 (slots q=0 / q=2), chunk b covers own tokens 512:1024 (q=1/q=3)
    arin = [nc.dram_tensor(f"arin_{i}", [2, C, 512], BF16) for i in range(2)]
    arout = [
        nc.dram_tensor(f"arout_{i}", [C, 512], BF16) for i in range(2)
    ]
    groups = [[2 * i, 2 * i + 1] for i in range(4)]

    for _rep in range(reps):
        with tile.TileContext(nc) as tc:
            with (
                tc.tile_pool(name="consts", bufs=1) as consts,
                tc.tile_pool(name="small", bufs=4) as small,
                tc.tile_pool(name="bc", bufs=3) as bc,
                tc.tile_pool(name="persist", bufs=1) as persist,
                tc.tile_pool(name="wfcp", bufs=1) as wfcp,
                tc.tile_pool(name="dramp", bufs=2, space="DRAM") as dramp,
            ):
                ones_b = consts.tile([128, 1], BF16, tag="ones", name="ones")
                nc.vector.memset(ones_b, 1.0)
                mask_sb = consts.tile([128, 128], F32, tag="mask", name="mask")
                nc.sync.dma_start(out=mask_sb, in_=maskT[:, :])
                bp_sb = consts.tile([128, NB], F32, tag="bp", name="bp")
                nc.sync.dma_start(out=bp_sb, in_=biasp[:, :])
                bv_b = consts.tile([128, 384], F32, tag="bvb", name="bvb")
                nc.sync.dma_start(out=bv_b, in_=_bcast_ap(bv[:], 128, 384))

                # x residual (own half) resident: [128, 6*1024] f32
                xh_all = persist.tile([128, CT * TH], F32, tag="xh", name="xh")

                # x2 = x + attn residual (own half), f32 resident
                # x2 = x + attn residual, bf16, c-chunk-major [c0 1024 | c1 ...]
                x2a = persist.tile([128, CT * TH], BF16, tag="x2a", name="x2a")

                def x2_ap(hn, width=512):
                    return bass.AP(
                        tensor=x2a.tensor,
                        offset=x2a.offset + hn * 512,
                        ap=[list(x2a.ap[0]), [TH, CT], [1, width]],
                    )


                def ln_stats(n_tag, stats_ps, xb_src, nsl, ab_dst):
                    """Emit stats for one 512-token chunk.

                    xb_src(c) -> bf16 [128, 512] slice of LN input chunk c.
                    ab_dst: bc-pool tile [128, 1024] bf16 receiving the
                    broadcast A (cols 0:512) and B (cols 512:1024).
                    """
                    ps = stats_ps.tile([33, 512], F32, tag="lnst", name="lnst")
                    for c in range(CT):
                        xbs = xb_src(c)
                        xs = small.tile(
                            [128, 512], BF16, tag="ln_xs", name="ln_xs", bufs=3
                        )
                        nc.vector.tensor_mul(out=xs, in0=xbs, in1=xbs)
                        nc.tensor.matmul(
                            ps[0:1, :], ones_b, xbs,
                            start=(c == 0), stop=(c == CT - 1),
                        )
                        nc.tensor.matmul(
                            ps[32:33, :], ones_b, xs,
                            start=(c == 0), stop=(c == CT - 1),
                        )
                    mean = small.tile([1, 512], F32, tag="ln_mean",
                                      name="ln_mean", bufs=2)
                    ex2 = small.tile([1, 512], F32, tag="ln_ex2", name="ln_ex2",
                                     bufs=2)
                    nc.scalar.activation(out=mean, in_=ps[0:1, :],
                                         func=AF.Copy, scale=1.0 / C)
                    nc.scalar.activation(out=ex2, in_=ps[32:33, :],
                                         func=AF.Copy, scale=1.0 / C)
                    var = small.tile([1, 512], F32, tag="ln_var", name="ln_var",
                                     bufs=2)
                    nc.vector.tensor_mul(out=var, in0=mean, in1=mean)
                    nc.vector.tensor_sub(out=var, in0=ex2, in1=var)
                    a_row = small.tile([1, 512], BF16, tag="ln_a", name="ln_a",
                                       bufs=2)
                    b_row = small.tile([1, 512], BF16, tag="ln_b", name="ln_b",
                                       bufs=2)
                    std = small.tile([1, 512], F32, tag="ln_std", name="ln_std",
                                     bufs=2)
                    nc.scalar.activation(out=std, in_=var, func=AF.Sqrt)
                    nc.vector.tensor_scalar_add(out=std, in0=std, scalar1=EPS)
                    with nc.allow_low_precision(reason="LN scale to bf16"):
                        nc.vector.reciprocal(out=a_row, in_=std)
                    # B = -mean * A
                    nc.vector.scalar_tensor_tensor(
                        out=b_row, in0=mean, scalar=-1.0,
                        in1=a_row, op0=ALU.mult, op1=ALU.mult,
                    )
                    dab = dramp.tile([2, 512], BF16, tag=f"d_ab{n_tag}",
                                     name=f"d_ab{n_tag}", bufs=2)
                    nc.sync.dma_start(out=dab[0:1, :], in_=a_row)
                    nc.sync.dma_start(out=dab[1:2, :], in_=b_row)
                    nc.sync.dma_start(out=ab_dst, in_=_bcast_ap(dab, 128, 1024))

                def ln_apply(scratch, xb_slice, ab_b, out_t):
                    """out_t[128,512] bf16 = xb*A + B."""
                    t1 = scratch.tile([128, 512], BF16, tag="ln_t1", name="ln_t1")
                    nc.vector.tensor_mul(out=t1, in0=xb_slice, in1=ab_b[:, 0:512])
                    nc.vector.tensor_add(out=out_t, in0=t1, in1=ab_b[:, 512:1024])

                # ============== Phase A: load x, LN1 stats, QKV ==============
                with tc.tile_pool(name="attin", bufs=1) as attin:
                    QT = [
                        attin.tile([128, T], BF16, tag=f"QT{c}", name=f"QT{c}")
                        for c in range(3)
                    ]
                    KT = [
                        attin.tile([128, T], BF16, tag=f"KT{c}", name=f"KT{c}")
                        for c in range(3)
                    ]
                    V = [
                        attin.tile([128, HL, 65], BF16, tag=f"V{t}", name=f"V{t}")
                        for t in range(T // 128)
                    ]
                    for t in range(T // 128):
                        nc.vector.memset(V[t][:, :, 64:65], 1.0)

                    with (
                        tc.tile_pool(name="xbp", bufs=1) as xbp,
                        tc.tile_pool(name="wqkvp", bufs=1) as wqkvp,
                        tc.tile_pool(name="h1p", bufs=_t["h1p"]) as h1p,
                        tc.tile_pool(name="sc1", bufs=_t["sc1"]) as sc1,
                        tc.tile_pool(name="st1_ps", bufs=2, space="PSUM") as st1_ps,
                        tc.tile_pool(name="qkv_ps", bufs=_t["qkv_ps"], space="PSUM") as qkv_ps,
                        tc.tile_pool(name="v_ps", bufs=_t["v_ps"], space="PSUM") as v_ps,
                    ):
                        # resident bf16 x (full row), loaded directly
                        xb = [
                            xbp.tile([128, T], BF16, tag=f"xb{c}", name=f"xb{c}")
                            for c in range(CT)
                        ]
                        for c in range(CT):
                            nc.sync.dma_start(
                                out=xb[c], in_=xTb[c * 128 : (c + 1) * 128, :]
                            )
                        wqkv_sb = [
                            wqkvp.tile([128, 1152], BF16, tag=f"wqkv{c}",
                                       name=f"wqkv{c}")
                            for c in range(CT)
                        ]
                        for c in range(CT):
                            nc.sync.dma_start(
                                out=wqkv_sb[c],
                                in_=Wqkv[c * 128 : (c + 1) * 128, :],
                            )

                        nc.sync.dma_start(
                            out=xh_all, in_=_rowgrp_ap(xh[:, :], 128, CT, TH)
                        )
                        ab1 = [
                            bc.tile([128, 1024], BF16, tag=f"ab1_{n}",
                                    name=f"ab1_{n}", bufs=1)
                            for n in range(QC)
                        ]

                        def emit_stats(n):
                            nsl = slice(n * 512, (n + 1) * 512)
                            ln_stats(
                                "1", st1_ps, lambda c: xb[c][:, nsl], nsl, ab1[n]
                            )

                        def emit_qkv(n):
                            nsl = slice(n * 512, (n + 1) * 512)
                            h1c = []
                            for c in range(CT):
                                hh = h1p.tile([128, 512], BF16, tag=f"h1c{c}",
                                              name=f"h1c{c}")
                                ln_apply(sc1, xb[c][:, nsl], ab1[n], hh)
                                h1c.append(hh)
                            for bi, dst in ((0, QT), (1, KT)):
                                for oc in range(3):
                                    ps = qkv_ps.tile([128, 512], F32, tag="qkv",
                                                     name="qkv")
                                    for c in range(CT):
                                        nc.tensor.matmul(
                                            ps,
                                            wqkv_sb[c][
                                                :, bi * 384 + oc * 128
                                                : bi * 384 + (oc + 1) * 128
                                            ],
                                            h1c[c],
                                            start=(c == 0),
                                            stop=(c == CT - 1),
                                        )
                                    nc.scalar.activation(
                                        out=dst[oc][:, nsl], in_=ps,
                                        func=AF.Identity,
                                        bias=bp_sb[:, (BP_BQ, BP_BK)[bi] + oc : (BP_BQ, BP_BK)[bi] + oc + 1],
                                    )
                            for tl in range(4):
                                t = n * 4 + tl
                                ps = v_ps.tile([128, 384], F32, tag="vps",
                                               name="vps")
                                for c in range(CT):
                                    nc.tensor.matmul(
                                        ps,
                                        h1c[c][:, tl * 128 : (tl + 1) * 128],
                                        wqkv_sb[c][:, 768:1152],
                                        start=(c == 0),
                                        stop=(c == CT - 1),
                                    )
                                nc.vector.tensor_add(
                                    out=V[t][:, :, 0:64],
                                    in0=ps.rearrange("p (h d) -> p h d", h=HL),
                                    in1=bv_b.rearrange("p (h d) -> p h d", h=HL),
                                )

                        # software pipeline: stats run ahead of qkv
                        emit_stats(0)
                        emit_stats(1)
                        emit_qkv(0)
                        emit_stats(2)
                        emit_stats(3)
                        emit_qkv(1)
                        emit_qkv(2)
                        emit_qkv(3)

                    # ============== Phase B: attention + proj + RS ==========
                    with (
                        tc.tile_pool(name="wpp", bufs=1) as wpp,
                        tc.tile_pool(name="scp", bufs=2) as scp,
                        tc.tile_pool(name="att_s_ps", bufs=_t["s_ps"], space="PSUM") as s_ps,
                        tc.tile_pool(name="att_o_ps", bufs=_t["o_ps"], space="PSUM") as o_ps,
                        tc.tile_pool(name="proj_ps", bufs=_t["p_ps"], space="PSUM") as p_ps,
                        tc.tile_pool(name="att_sc", bufs=_t["att_sc"]) as att_sc,
                        tc.tile_pool(name="yraw", bufs=2) as yraw_p,
                    ):
                        wp_sb = wpp.tile([128, 3 * C], BF16, tag="wp", name="wp")
                        nc.sync.dma_start(
                            out=wp_sb, in_=_rowgrp_ap(Wp[:, :], 128, 3, C)
                        )
                        wfc_sb = [
                            wfcp.tile([128, HID], BF16, tag=f"wfc{c}",
                                      name=f"wfc{c}")
                            for c in range(CT)
                        ]
                        for c in range(CT):
                            nc.sync.dma_start(
                                out=wfc_sb[c],
                                in_=Wfc[c * 128 : (c + 1) * 128, :],
                            )

                        yT = [
                            att_sc.tile([128, 512], BF16, tag=f"yT{c}",
                                        name=f"yT{c}", bufs=2)
                            for c in range(3)
                        ]

                        for q in (0, 2, 1, 3):
                            qsl = slice(q * 512, (q + 1) * 512)
                            nst = 4 * q + 4
                            rv = att_sc.tile([1, HL * 512], BF16, tag="rv",
                                             name="rv", bufs=2)
                            ypair = [
                                yraw_p.tile([128, 512], BF16, tag=f"yp{p}",
                                            name=f"yp{p}")
                                for p in range(HPAIR)
                            ]
                            for ht in range(HPAIR):
                                po_e = o_ps.tile([65, 512], F32, tag="poe",
                                                 name="poe")
                                po_o = o_ps.tile([65, 512], F32, tag="poo",
                                                 name="poo")
                                for st in range(nst):
                                    r = st - 4 * q
                                    qlo = 128 * r if r >= 0 else 0
                                    csl = slice(qlo, 512)
                                    osl = slice(512 + qlo, 1024)
                                    sp = s_ps.tile([128, 1024], F32, tag="sp",
                                                   name="sp")
                                    ssl = slice(st * 128, (st + 1) * 128)
                                    nc.tensor.matmul(
                                        sp[:, csl],
                                        KT[ht][0:64, ssl],
                                        QT[ht][0:64, q * 512 + qlo : (q + 1) * 512],
                                        start=True, stop=True,
                                    )
                                    nc.tensor.matmul(
                                        sp[:, osl],
                                        KT[ht][64:128, ssl],
                                        QT[ht][64:128, q * 512 + qlo : (q + 1) * 512],
                                        start=True, stop=True,
                                    )
                                    pt = att_sc.tile([128, 1024], BF16, tag="pt",
                                                     name="pt")
                                    if r >= 0:
                                        spv = sp[:, 128 * r :].rearrange(
                                            "p (g x) -> p g x", g=2, x=128,
                                            allow_nc=True,
                                        ) if False else bass.AP(
                                            tensor=sp.tensor,
                                            offset=sp.offset + 128 * r,
                                            ap=[list(sp.ap[0]), [512, 2],
                                                [1, 128]],
                                        )
                                        mk2 = bass.AP(
                                            tensor=mask_sb.tensor,
                                            offset=mask_sb.offset,
                                            ap=[list(mask_sb.ap[0]), [0, 2],
                                                [1, 128]],
                                        )
                                        nc.vector.tensor_add(
                                            out=spv, in0=spv, in1=mk2,
                                        )
                                    if r <= 0:
                                        nc.scalar.activation(
                                            out=pt, in_=sp, func=AF.Exp,
                                            scale=0.125,
                                        )
                                    else:
                                        spv = bass.AP(
                                            tensor=sp.tensor,
                                            offset=sp.offset + qlo,
                                            ap=[list(sp.ap[0]), [512, 2],
                                                [1, 512 - qlo]],
                                        )
                                        ptv = bass.AP(
                                            tensor=pt.tensor,
                                            offset=pt.offset + qlo,
                                            ap=[list(pt.ap[0]), [512, 2],
                                                [1, 512 - qlo]],
                                        )
                                        nc.scalar.activation(
                                            out=ptv, in_=spv,
                                            func=AF.Exp, scale=0.125,
                                        )
                                    nc.tensor.matmul(
                                        po_e[:, csl],
                                        V[st][:, 2 * ht, :],
                                        pt[:, csl],
                                        start=(st == 0), stop=(st == nst - 1),
                                    )
                                    nc.tensor.matmul(
                                        po_o[:, csl],
                                        V[st][:, 2 * ht + 1, :],
                                        pt[:, osl],
                                        start=(st == 0), stop=(st == nst - 1),
                                    )
                                for par, po in ((0, po_e), (1, po_o)):
                                    h = 2 * ht + par
                                    with nc.allow_low_precision(
                                        reason="softmax denom recip to bf16"
                                    ):
                                        nc.vector.reciprocal(
                                            out=rv[0:1, h * 512 : (h + 1) * 512],
                                            in_=po[64:65, :],
                                        )
                                    nc.vector.tensor_copy(
                                        out=ypair[ht][par * 64 : par * 64 + 64, :],
                                        in_=po[0:64, :],
                                    )
                            drv = dramp.tile([1, HL * 512], BF16, tag="d_rv",
                                             name="d_rv", bufs=2)
                            nc.sync.dma_start(out=drv, in_=rv)
                            rb = att_sc.tile([128, HL * 512], BF16, tag="rb",
                                             name="rb", bufs=2)
                            nc.sync.dma_start(
                                out=rb, in_=_bcast_ap(drv, 128, HL * 512)
                            )
                            for ht in range(HPAIR):
                                for par in range(2):
                                    h = 2 * ht + par
                                    psl = slice(par * 64, par * 64 + 64)
                                    nc.vector.tensor_mul(
                                        out=yT[ht][psl, :],
                                        in0=ypair[ht][psl, :],
                                        in1=rb[psl, h * 512 : (h + 1) * 512],
                                    )
                            approj = scp.tile([128, CT * 512], BF16, tag="apj",
                                              name="apj")
                            for oc in range(CT):
                                ps = p_ps.tile([128, 512], F32, tag="pps",
                                               name="pps")
                                for ci in range(3):
                                    nc.tensor.matmul(
                                        ps,
                                        wp_sb[:, ci * C + oc * 128
                                              : ci * C + (oc + 1) * 128],
                                        yT[ci],
                                        start=(ci == 0), stop=(ci == 2),
                                    )
                                nc.vector.tensor_scalar_add(
                                    out=approj[:, oc * 512 : (oc + 1) * 512],
                                    in0=ps,
                                    scalar1=bp_sb[:, BP_BAP + oc : BP_BAP + oc + 1],
                                )
                            nc.sync.dma_start(
                                out=_rowgrp_ap(arin[q % 2][q // 2, :, :], 128, CT, 512),
                                in_=approj,
                            )
                            if q == 2 or q == 3:
                                i = q % 2
                                if fake_cc:
                                    nc.sync.dma_start(
                                        out=arout[i][:, :], in_=arin[i][0, :, :]
                                    )
                                else:
                                    nc.gpsimd.collective_compute(
                                        "ReduceScatter",
                                        ALU.add,
                                        replica_groups=groups,
                                        ins=[arin[i][:, :, :]],
                                        outs=[arout[i][:, :]],
                                    )

                # ========= Phase C/D: residual + LN2 + FFN per half-chunk ====
                with (
                    tc.tile_pool(name="ffnw", bufs=1) as ffnw,
                    tc.tile_pool(name="gtp", bufs=1) as gtp,
                    tc.tile_pool(name="scr", bufs=2) as scr,
                    tc.tile_pool(name="sc3", bufs=3) as sc3,
                    tc.tile_pool(name="st2_ps", bufs=2, space="PSUM") as st2_ps,
                    tc.tile_pool(name="fc_ps", bufs=_t["fc_ps"], space="PSUM") as fc_ps,
                    tc.tile_pool(name="mp_ps", bufs=_t["mp_ps"], space="PSUM") as mp_ps,
                ):
                    # wmp_sb[g] holds m-chunks 4g..4g+3 side by side
                    wmp_sb = [
                        ffnw.tile([128, 4 * C], BF16, tag=f"wmp{g}", name=f"wmp{g}")
                        for g in range(6)
                    ]
                    for g in range(6):
                        nc.sync.dma_start(
                            out=wmp_sb[g],
                            in_=_rowgrp_ap(
                                Wmp[g * 512 : (g + 1) * 512, :], 128, 4, C
                            ),
                        )
                    gT = [
                        gtp.tile([128, 512], BF16, tag=f"gT{m}", name=f"gT{m}")
                        for m in range(HCT)
                    ]
                    for hn in range(2):
                        nsl = slice(hn * 512, (hn + 1) * 512)
                        att_all = scr.tile([128, CT * 512], BF16, tag="attall",
                                           name="attall")
                        nc.sync.dma_start(
                            out=att_all,
                            in_=_rowgrp_ap(arout[hn][:, :], 128, CT, 512),
                        )
                        xh_v = bass.AP(
                            tensor=xh_all.tensor,
                            offset=xh_all.offset + hn * 512,
                            ap=[list(xh_all.ap[0]), [TH, CT], [1, 512]],
                        )
                        nc.vector.tensor_add(
                            out=x2_ap(hn), in0=xh_v, in1=att_all,
                        )
                        ab2 = bc.tile([128, 1024], BF16, tag="ab2", name="ab2")
                        ln_stats(
                            "2", st2_ps,
                            lambda c, _n=nsl: x2a[:, c * TH + _n.start
                                                  : c * TH + _n.stop],
                            nsl, ab2,
                        )
                        h2c = []
                        for c in range(CT):
                            hh = sc3.tile([128, 512], BF16, tag=f"h2c{c}",
                                          name=f"h2c{c}", bufs=2)
                            ln_apply(
                                sc3,
                                x2a[:, c * TH + nsl.start : c * TH + nsl.stop],
                                ab2, hh,
                            )
                            h2c.append(hh)
                        for m in range(HCT):
                            ps = fc_ps.tile([128, 512], F32, tag="fps",
                                            name="fps")
                            for c in range(CT):
                                nc.tensor.matmul(
                                    ps,
                                    wfc_sb[c][:, m * 128 : (m + 1) * 128],
                                    h2c[c],
                                    start=(c == 0), stop=(c == CT - 1),
                                )
                            nc.scalar.activation(
                                out=gT[m], in_=ps, func=AF.Gelu,
                                bias=bp_sb[:, BP_BFC + m : BP_BFC + m + 1],
                            )
                        for oc in range(CT):
                            ps = mp_ps.tile([128, 512], F32, tag="mps",
                                            name="mps")
                            for m in range(HCT):
                                nc.tensor.matmul(
                                    ps,
                                    wmp_sb[m // 4][
                                        :, (m % 4) * C + oc * 128
                                        : (m % 4) * C + (oc + 1) * 128
                                    ],
                                    gT[m],
                                    start=(m == 0), stop=(m == HCT - 1),
                                )
                            mp = sc3.tile([128, 512], F32, tag="mp_ev",
                                          name="mp_ev", bufs=2)
                            nc.scalar.activation(
                                out=mp, in_=ps, func=AF.Identity,
                                bias=bp_sb[:, BP_BMP + oc : BP_BMP + oc + 1],
                            )
                            o = sc3.tile([128, 512], F32, tag="r2o",
                                         name="r2o", bufs=2)
                            nc.vector.tensor_add(
                                out=o,
                                in0=x2a[:, oc * TH + nsl.start
                                        : oc * TH + nsl.stop],
                                in1=mp,
                            )
                            nc.sync.dma_start(
                                out=outT[oc * 128 : (oc + 1) * 128, nsl], in_=o
                            )

    nc.finalize()
    return nc


# ---------------------------------------------------------------------------
_RUNNER = {}
_NC = None


def _get_nc():
    global _NC
    if _NC is None:
        _NC = build_nc()
    return _NC


def _make_runner(chain=1, nc=None):
    import jax
    from jax.sharding import Mesh, PartitionSpec
    from jax.experimental.shard_map import shard_map
    from concourse import bass2jax

    if nc is None:
        nc = _get_nc()
    bass2jax.install_neuronx_cc_hook()

    partition_name = (
        nc.partition_id_tensor.name if nc.partition_id_tensor else None
    )
    in_names, out_names, out_avals, zero_outs = [], [], [], []
    for alloc in nc.m.functions[0].allocations:
        if not isinstance(alloc, mybir.MemoryLocationSet):
            continue
        name = alloc.memorylocations[0].name
        if alloc.kind == "ExternalInput":
            if name != partition_name:
                in_names.append(name)
        elif alloc.kind == "ExternalOutput":
            shape = tuple(alloc.tensor_shape)
            dtype = mybir.dt.np(alloc.dtype)
            out_names.append(name)
            out_avals.append(jax.core.ShapedArray(shape, dtype))
            zero_outs.append(np.zeros(shape, dtype))
    n_params = len(in_names)
    n_outs = len(out_avals)
    all_names = in_names + out_names
    if partition_name is not None:
        all_names = all_names + [partition_name]
    donate = tuple(range(n_params, n_params + n_outs))

    def _body(*args):
        operands = list(args)
        if partition_name is not None:
            operands.append(bass2jax.partition_id_tensor())
        outs = bass2jax._bass_exec_p.bind(
            *operands,
            out_avals=tuple(out_avals),
            in_names=tuple(all_names),
            out_names=tuple(out_names),
            lowering_input_output_aliases=(),
            sim_require_finite=True,
            sim_require_nnan=True,
            nc=nc,
        )
        return tuple(outs)

    devices = jax.devices()[:N_CORES]
    mesh = Mesh(np.asarray(devices), ("core",))
    in_specs = (PartitionSpec("core"),) * (n_params + n_outs)
    out_specs = (PartitionSpec("core"),) * n_outs
    sharded = jax.jit(
        shard_map(
            _body, mesh=mesh, in_specs=in_specs, out_specs=out_specs, check_rep=False
        ),
        donate_argnums=donate,
        keep_unused=True,
    )
    return sharded, in_names, out_names, out_avals, zero_outs


def get_runner(chain=1):
    if chain not in _RUNNER:
        _RUNNER[chain] = _make_runner(chain)
    return _RUNNER[chain]


def make_core_inputs(
    x, ln1_w, ln1_b, W_attn, b_attn, W_attn_proj, b_attn_proj,
    ln2_w, ln2_b, W_fc, b_fc, W_mlp_proj, b_mlp_proj,
):
    """Host-side sharding: returns list of 8 dicts of per-core numpy arrays."""
    bf = ml_dtypes.bfloat16
    x = np.asarray(x, np.float32)
    ln1_w = np.asarray(ln1_w, np.float32)
    ln1_b = np.asarray(ln1_b, np.float32)
    ln2_w = np.asarray(ln2_w, np.float32)
    ln2_b = np.asarray(ln2_b, np.float32)
    W_attn = np.asarray(W_attn, np.float32)
    b_attn = np.asarray(b_attn, np.float32)
    W_fc = np.asarray(W_fc, np.float32)
    b_fc = np.asarray(b_fc, np.float32)

    srow, scol = np.meshgrid(np.arange(128), np.arange(128), indexing="ij")
    maskT = np.where(srow <= scol, 0.0, NEG).astype(np.float32)

    # fold LN1 gain into W_attn, LN1 bias into b_attn
    Wattn_f = W_attn * ln1_w[:, None]
    battn_f = b_attn + ln1_b @ W_attn
    # fold LN2 gain into W_fc, bias into b_fc
    Wfc_f = (W_fc * ln2_w[:, None]).astype(bf)
    bfc_f = b_fc + ln2_b @ W_fc
    wmp_bf = np.ascontiguousarray(W_mlp_proj).astype(bf)

    core_ins = []
    for core in range(N_CORES):
        b, par = core // 2, core % 2
        hs = slice(par * 384, (par + 1) * 384)
        ks = slice(C + par * 384, C + (par + 1) * 384)
        vs = slice(2 * C + par * 384, 2 * C + (par + 1) * 384)
        xt = np.ascontiguousarray(x[b].T)
        xtb = xt.astype(bf)

        Wqkv = np.concatenate(
            [Wattn_f[:, hs], Wattn_f[:, ks], Wattn_f[:, vs]], axis=1
        ).astype(bf)

        biasp = np.zeros((128, NB), np.float32)
        biasp[:, BP_BQ : BP_BQ + 3] = battn_f[hs].reshape(3, 128).T
        biasp[:, BP_BK : BP_BK + 3] = battn_f[ks].reshape(3, 128).T
        biasp[:, BP_BAP : BP_BAP + 6] = (
            np.asarray(b_attn_proj, np.float32).reshape(6, 128).T / 2
        )
        biasp[:, BP_BMP : BP_BMP + 6] = (
            np.asarray(b_mlp_proj, np.float32).reshape(6, 128).T
        )
        biasp[:, BP_BFC : BP_BFC + 24] = bfc_f.reshape(24, 128).T

        core_ins.append(
            dict(
                xTb=xtb,
                xh=np.ascontiguousarray(xt[:, par * TH : (par + 1) * TH]),
                Wqkv=Wqkv,
                Wp=np.ascontiguousarray(W_attn_proj[hs, :]).astype(bf),
                Wfc=Wfc_f,
                Wmp=wmp_bf,
                biasp=biasp,
                bv=battn_f[vs].astype(np.float32),
                maskT=maskT,
            )
        )
    return core_ins


def run_cores(core_ins):
    """Execute the SPMD program; returns [N_CORES, C, TH] stacked outT."""
    sharded, in_names, out_names, out_avals, zero_outs = get_runner()
    concat_in = [
        np.concatenate([np.asarray(core_ins[c][n]) for c in range(N_CORES)], axis=0)
        for n in in_names
    ]
    concat_zeros = [
        np.zeros((N_CORES * z.shape[0], *z.shape[1:]), z.dtype) for z in zero_outs
    ]
    outs = sharded(*concat_in, *concat_zeros)
    return np.asarray(outs[0]).reshape(N_CORES, C, TH)


def kernel(**inputs):
    core_ins = make_core_inputs(**inputs)
    o = run_cores(core_ins)
    out = np.empty((B, T, C), np.float32)
    for b in range(B):
        out[b, 0:TH] = o[2 * b].T
        out[b, TH:] = o[2 * b + 1].T
    return out
